# revision 68
# baseline (speedup 1.0000x reference)
"""Batched causal self-attention (B=4, T=2048, C=1024, H=16) on 8 trn2 NeuronCores.

Sharding: data-parallel over B (4) x tensor-parallel over head-halves (2).
Core c handles batch b=c//2, heads [hh*8, hh*8+8) with hh=c%2. Each core
computes its qkv projection slice, causal attention for its 8 heads, and a
partial output projection (512 rows of W_proj); the host sums the two
partials per batch (the TP all-reduce).

Per-core pipeline (bf16 front-end):
  phase 0: a few matmuls on zeroed SBUF warm the PE clock gate (HAM) under
           the initial DMA shadow.
  phase 1: v = x @ Wv -> v_aug [128, 16, 8*128] bf16 (per-head block: 64
           v-dims | ones col for the softmax denominator | uninitialized pad
           that only feeds never-read AV output rows; 128-col weights keep
           the fast weight load path). Only v blocks 0-7 / key chunks 0-1 /
           query chunk 0 are computed up front — the rest is paced into pair
           0's superslots so attention starts while the xT DMA tail lands.
  phase 2 (per head-pair): q^T = Wq-pair.T @ x^T [128, T]; k^T stored
           zero-padded per head (head A keys in rows 0:64, head B in rows
           64:128, other half zero) so every S matmul runs full K=128 —
           mixed K=64/K=128 streams stall the PE weight loader on row-group
           conflicts. Per 512-wide query chunk, superslots of two key
           blocks: S^T = kpad-block.T @ qT per head into a 2-bank PSUM tile;
           band (diagonal) superslots write their live ranges PACKED so one
           exp instruction per head covers both blocks; exp on ACT
           (scale=1/8 folded; no max-subtraction needed, scores
           ~N(0,0.4^2)); only the 128-wide diagonal sub-block needs the
           causal triangle -> in-place [128,128] mask multiply; AV psum
           accumulation runs 2 superslots behind S so its wait on exp is
           satisfied when the PE reaches it; fully-masked leading queries of
           band blocks are skipped end-to-end (S, exp, AV).
           Engines run their queues in order, so independent PE work (the
           rest of phase 1, the next pair's qk projection, the output
           projection for finished query chunks) is interleaved via a paced
           backlog to fill the exp shadow and pipeline-fill bubbles; PSUM is
           split 4/2/2 banks: S scores / proj+vproj scratch / AV, keeping
           the projection scratch out of the S-score WAR rotation.
  phase 3: remaining output projection tail; the last chunk normalizes
           straight from PSUM to shorten the critical tail.
"""

import numpy as np
import ml_dtypes

import concourse.bass as bass
import concourse.mybir as mybir
import concourse.tile as tile
from concourse import bacc
from concourse.bass import ds, ts
from concourse.bass_utils import run_bass_kernel_spmd

B, T, C, H = 4, 2048, 1024, 16
D = 64
NCORES = 8
NPAIR = 4              # head pairs per core (8 heads)
NK = C // 128          # 8 contraction tiles over C
NT = T // 128          # 16 tiles over T
NCH = T // 512         # 4 query chunks
INV_SCALE = 0.125      # 1 / sqrt(C // H)

f32 = mybir.dt.float32
f32r = mybir.dt.float32r
bf16 = mybir.dt.bfloat16
fp16 = mybir.dt.float16

_cache = {}
LAST_RESULTS = None    # test harness reads exec_time_ns from here

# If the caller sets BASS_TRACE=1, run_bass_kernel_spmd imports
# antenv.axon_hooks, which some container images don't ship. Provide a stub
# so tracing degrades gracefully instead of raising ImportError.
try:
    import antenv.axon_hooks  # noqa: F401
except ImportError:
    import sys as _sys
    import types as _types

    _m = _types.ModuleType("antenv.axon_hooks")
    _m._hook = None
    _m.set_axon_ntff_profile_hook = lambda h: setattr(_m, "_hook", h)
    _m.get_axon_ntff_profile_hook = lambda: _m._hook
    _sys.modules["antenv.axon_hooks"] = _m
    # Boot's own registration is skipped when the image's antenv lacks
    # axon_hooks; re-register the ctypes NTFF hook against the injected
    # axon .so so traces + exec_time_ns work. Best-effort.
    try:
        from trn_agent_boot.trn_boot import _ntff_profile_via_ctypes

        _m.set_axon_ntff_profile_hook(
            _ntff_profile_via_ctypes("/opt/axon/libaxon_pjrt.so"))
    except Exception:
        pass


def _build():
    nc = bacc.Bacc("TRN2", target_bir_lowering=False, debug=False)
    xT_d = nc.dram_tensor("xT", [C, T], bf16, kind="ExternalInput").ap()
    wqk_d = nc.dram_tensor("wqk", [8, NK, 128, 128], bf16, kind="ExternalInput").ap()
    wv_d = nc.dram_tensor("wv", [C, 512], bf16, kind="ExternalInput").ap()
    wp_d = nc.dram_tensor("wp", [512, C], fp16, kind="ExternalInput").ap()
    mask_d = nc.dram_tensor("mask", [128, 128], bf16, kind="ExternalInput").ap()
    out_d = nc.dram_tensor("out", [T, C], f32, kind="ExternalOutput").ap()

    Exp = mybir.ActivationFunctionType.Exp

    with tile.TileContext(nc) as tc:
        with tc.tile_pool(name="persist", bufs=1) as persist:
            # per-head blocks of 128 cols (v[0:64] | ones at 64 | garbage):
            # the ones column produces the softmax denominator inside the AV
            # matmul; cols 65-127 are never initialized — they only feed AV
            # output rows 65-127, which are never read. 128-col weights keep
            # the fast weight load path.
            v_aug = persist.tile([128, NT, 8 * 128], bf16, tag="vaug")
            yT = persist.tile([128, NPAIR, T], fp16, tag="yT")
            mask_t = persist.tile([128, 128], bf16, tag="mask")
            wp_t = persist.tile([128, 4, C], fp16, tag="wp")
            # zero-padded per-head key projections: head A keys in rows 0:64
            # (rows 64:128 zero), head B in rows 64:128 (rows 0:64 zero), so
            # S matmuls run full K=128 — mixed K=64/K=128 instruction streams
            # stall the PE weight loads on row-group conflicts.
            kpads = persist.tile([128, NPAIR, 2, T], bf16, tag="kpads")

            with (
                tc.tile_pool(name="xpool", bufs=1) as xpool,
                tc.tile_pool(name="wvpool", bufs=1) as wvpool,
                tc.tile_pool(name="wqkpool", bufs=2) as wqkpool,
                tc.tile_pool(name="qkpool", bufs=2) as qkpool,
                tc.tile_pool(name="epool", bufs=8) as epool,
                tc.tile_pool(name="npool", bufs=2) as npool,
                tc.tile_pool(name="opool", bufs=4) as opool,
                tc.tile_pool(name="spsum", bufs=2, space="PSUM") as spsum,
                tc.tile_pool(name="ppsum", bufs=2, space="PSUM") as ppsum,
                tc.tile_pool(name="avpsum", bufs=2, space="PSUM") as avpsum,
            ):
                xT_t = xpool.tile([128, NK, T], bf16)

                # ---- phase 0: PE warm-up under the initial DMA shadow ----
                # ~30 matmuls on uninitialized SBUF (results never read) keep
                # the PE busy from t~1us so the HAM clock gate reaches 8/8
                # before the real work starts; otherwise the first ~10us of
                # matmuls run at half clock.
                warm_w = xpool.tile([128, 128 + 512], bf16, tag="warm")
                nc.vector.memset(warm_w.bitcast(mybir.dt.uint16), 0)
                warm_ps = spsum.tile([128, 512], f32, tag="s", name="warmps")
                for wi in range(8):
                    nc.tensor.matmul(warm_ps[:], warm_w[:, 0:128],
                                     warm_w[:, 128:640],
                                     start=True, stop=True)

                # ---- phase 1: DMA issue + early v/qk projection ----
                # xT arrives in per-(k, t-chunk) pieces; attention on pair 0
                # chunk 0 needs only xT chunks 0-1 (+ wv + wpair0), so it
                # starts while the xT tail is still in flight. The remaining
                # v-proj / qk-proj groups are paced into pair 0's backlog.
                wv_t = wvpool.tile([128, NK, 512], bf16)
                va4 = v_aug.rearrange("p n (h e) -> p n h e", e=128)
                qk_tiles = {}

                def start_pair_w(p):
                    def go():
                        wpair = wqkpool.tile([128, NK, 256], bf16, tag="w",
                                             name=f"wpair{p}")
                        for m2 in range(2):
                            nc.sync.dma_start(
                                wpair[:, :, ds(m2 * 128, 128)],
                                wqk_d[4 * m2 + p].rearrange("ko p m -> p ko m"))
                        q_t = qkpool.tile([128, T], bf16, tag="q", name=f"q{p}")
                        qk_tiles[p] = (q_t, wpair)
                        # zero halves of this pair's padded key tiles — on
                        # gpsimd, which is idle, so the DVE queue stays clear
                        nc.gpsimd.memset(
                            kpads[D:128, p, 0].bitcast(mybir.dt.uint16), 0)
                        nc.gpsimd.memset(
                            kpads[0:D, p, 1].bitcast(mybir.dt.uint16), 0)
                    return go

                def vproj_group(t):
                    def go():
                        ps = ppsum.tile([128, 512], f32, tag="pp",
                                        name=f"vps{t}")
                        for k in range(NK):
                            nc.tensor.matmul(
                                ps[:], xT_t[:, k, ts(t, 128)], wv_t[:, k],
                                start=(k == 0), stop=(k == NK - 1),
                            )
                        nc.vector.tensor_copy(
                            va4[:, t, :, 0:D],
                            ps.rearrange("p (h d) -> p h d", d=D))
                    return go

                for k in range(NK):
                    nc.sync.dma_start(xT_t[:, k, 0:512],
                                      xT_d[ds(k * 128, 128), 0:512])
                start_pair_w(0)()
                for k in range(NK):
                    nc.sync.dma_start(wv_t[:, k], wv_d[ds(k * 128, 128)])
                for k in range(NK):
                    nc.sync.dma_start(xT_t[:, k, ds(512, 512)],
                                      xT_d[ds(k * 128, 128), ds(512, 512)])
                nc.sync.dma_start(mask_t[:], mask_d)
                for tc4 in range(2, 4):
                    for k in range(NK):
                        nc.sync.dma_start(
                            xT_t[:, k, ds(tc4 * 512, 512)],
                            xT_d[ds(k * 128, 128), ds(tc4 * 512, 512)])
                for kp in range(4):
                    nc.sync.dma_start(wp_t[:, kp], wp_d[ds(kp * 128, 128)])

                nc.vector.memset(
                    va4[:, :, :, D:D + 1].bitcast(mybir.dt.uint16), 0x3F80)

                def qkproj_group(p, m2, n):
                    def go():
                        q_t, wpair = qk_tiles[p]
                        ps = ppsum.tile([128, 512], f32, tag="pp",
                                        name=f"qkps{p}_{m2}_{n}")
                        for k in range(NK):
                            nc.tensor.matmul(
                                ps[:], wpair[:, k, ds(m2 * 128, 128)],
                                xT_t[:, k, ds(n * 512, 512)],
                                start=(k == 0), stop=(k == NK - 1))
                        if m2 == 0:
                            nc.vector.tensor_copy(q_t[:, ds(n * 512, 512)], ps[:])
                        else:
                            nc.vector.tensor_copy(
                                kpads[0:D, p, 0, ds(n * 512, 512)], ps[0:D, :])
                            nc.vector.tensor_copy(
                                kpads[D:128, p, 1, ds(n * 512, 512)], ps[D:128, :])
                    return go

                o_tiles = {}

                def proj_group(t, n2):
                    def go():
                        o_t = opool.tile([128, 512], f32, tag="o",
                                         name=f"o{t}_{n2}")
                        ps = ppsum.tile([128, 512], f32, tag="pp",
                                        name=f"pps{t}_{n2}")
                        for kp in range(4):
                            nc.tensor.matmul(
                                ps[:], yT[:, kp, ts(t, 128)],
                                wp_t[:, kp, ds(n2 * 512, 512)],
                                start=(kp == 0), stop=(kp == 3))
                        nc.vector.tensor_copy(o_t[:], ps[:])
                        nc.sync.dma_start(
                            out_d[ds(t * 128, 128), ds(n2 * 512, 512)], o_t[:])
                    return go

                # phase 1a: ONLY pair-0 chunk-0's critical chain (keys +
                # queries chunk 0, v blocks 0-1) so the S/exp/AV pipeline
                # starts ~15us in and chunk-0 attention work fills the xT
                # chunk 1-3 DMA waits. Everything else is paced into pair
                # 0's superslots in dependency order (S(c) needs key chunks
                # <= c and q chunk c; AV(c) needs v blocks <= 4c+3).
                qkproj_group(0, 1, 0)()
                qkproj_group(0, 0, 0)()
                vproj_group(0)()
                vproj_group(1)()
                pre_backlog = [vproj_group(2), vproj_group(3),
                               qkproj_group(0, 1, 1), qkproj_group(0, 0, 1)]
                pre_backlog += [vproj_group(t) for t in range(4, 8)]
                pre_backlog += [qkproj_group(0, 1, 2), qkproj_group(0, 0, 2)]
                pre_backlog += [vproj_group(t) for t in range(8, 12)]
                pre_backlog += [qkproj_group(0, 1, 3), qkproj_group(0, 0, 3)]
                pre_backlog += [vproj_group(t) for t in range(12, 16)]

                deferred_norm = []

                def make_norm(avs, head, p, c):
                    def go():
                        dn = npool.tile([1, 512], f32, tag="dn",
                                        name=f"dn{p}_{c}_{head}")
                        nc.vector.tensor_copy(dn[:], avs[D:D + 1, :])
                        rr = npool.tile([1, 512], f32, tag="rr",
                                        name=f"rr{p}_{c}_{head}")
                        nc.vector.reciprocal_approx_fast(out=rr[:], in_=dn[:])
                        rb = npool.tile([D, 512], f32, tag="rb",
                                        name=f"rb{p}_{c}_{head}")
                        nc.gpsimd.partition_broadcast(rb[:], rr[:])
                        nc.vector.tensor_mul(
                            yT[ds(D * head, D), p, ds(c * 512, 512)],
                            avs[0:D, :], rb[:])
                    return go

                # ---- phase 2: attention per pair, with PE backlog interleave ----
                # two m2=0 (query-side) qkproj groups of the last pair are held
                # back to fill the last pair's own chunk-0 pipeline bubble
                held_for_last = []
                for p in range(NPAIR):
                    backlog = []
                    if p == 0:
                        backlog.extend(pre_backlog)
                    if p == NPAIR - 1:
                        backlog.extend(held_for_last)
                    if p + 1 < NPAIR:
                        backlog.append(start_pair_w(p + 1))
                        for m2 in range(2):
                            for n in range(NCH):
                                g = qkproj_group(p + 1, m2, n)
                                if p + 1 == NPAIR - 1 and m2 == 0 and n >= 2:
                                    held_for_last.append(g)
                                else:
                                    backlog.append(g)
                    q_t, _ = qk_tiles[p]
                    total_slots = sum(2 * (cc + 1) + 3 for cc in range(NCH))
                    done_slots = 0
                    emitted = 0
                    for c in range(NCH):
                        if p == NPAIR - 1 and c >= 1:
                            # projection for query chunk c-1 is complete
                            for t in range(4 * (c - 1), 4 * c):
                                for n2 in range(2):
                                    backlog.append(proj_group(t, n2))
                        nblk = 4 * (c + 1)
                        nsuper = nblk // 2
                        av_A = avpsum.tile([128, 512], f32, tag="av",
                                           name=f"avA{p}_{c}")
                        av_B = avpsum.tile([128, 512], f32, tag="av",
                                           name=f"avB{p}_{c}")
                        pend = {}
                        drain_at = 0 if p == NPAIR - 1 else 1
                        for s in range(nsuper + 3):
                            if s == drain_at and deferred_norm:
                                for fn in deferred_norm:
                                    fn()
                                deferred_norm.clear()
                            if s < nsuper:
                                i = s - (nsuper - 2)
                                sA = spsum.tile([128, 1024], f32, tag="s",
                                                name=f"sA{p}_{c}_{s}")
                                sB = spsum.tile([128, 1024], f32, tag="s",
                                                name=f"sB{p}_{c}_{s}")
                                if i < 0:
                                    # layout: (tile col offset, width, query
                                    # offset within chunk) per key block
                                    lay = [(0, 512, 0), (512, 512, 0)]
                                else:
                                    # band: queries [0, 128d) are fully masked
                                    # — compute only the live range, PACKED so
                                    # one exp instruction covers both halves
                                    w0 = 512 - 256 * i
                                    lay = [(0, w0, 256 * i),
                                           (w0, w0 - 128, 256 * i + 128)]
                                for half in (0, 1):
                                    tj = 2 * s + half
                                    rd, w, qo = lay[half]
                                    nc.tensor.matmul(
                                        sA[:, ds(rd, w)],
                                        kpads[:, p, 0, ts(tj, 128)],
                                        q_t[:, ds(c * 512 + qo, w)],
                                        start=True, stop=True)
                                    nc.tensor.matmul(
                                        sB[:, ds(rd, w)],
                                        kpads[:, p, 1, ts(tj, 128)],
                                        q_t[:, ds(c * 512 + qo, w)],
                                        start=True, stop=True)
                                e_A = epool.tile([128, 1024], bf16, tag="e",
                                                 name=f"eA{p}_{c}_{s}")
                                e_B = epool.tile([128, 1024], bf16, tag="e",
                                                 name=f"eB{p}_{c}_{s}")
                                W = lay[1][0] + lay[1][1]
                                nc.scalar.activation(e_A[:, 0:W], sA[:, 0:W], Exp,
                                                     scale=INV_SCALE)
                                nc.scalar.activation(e_B[:, 0:W], sB[:, 0:W], Exp,
                                                     scale=INV_SCALE)
                                if i >= 0:
                                    # only the 128-wide diagonal sub-block of
                                    # each band block has a causal triangle ->
                                    # in-place mask there
                                    for half in (0, 1):
                                        dsl = ds(lay[half][0], 128)
                                        nc.vector.tensor_mul(e_A[:, dsl], e_A[:, dsl],
                                                             mask_t[:])
                                        nc.vector.tensor_mul(e_B[:, dsl], e_B[:, dsl],
                                                             mask_t[:])
                                pend[s] = (e_A, e_B, lay)
                            if s >= 3:
                                e_A, e_B, lay2 = pend.pop(s - 3)
                                s2 = s - 3
                                for half in (0, 1):
                                    tj = 2 * s2 + half
                                    rd, w, qo = lay2[half]
                                    nc.tensor.matmul(
                                        av_A[:, ds(qo, w)],
                                        va4[:, tj, 2 * p],
                                        e_A[:, ds(rd, w)],
                                        start=(tj == 0), stop=(tj == nblk - 1))
                                    nc.tensor.matmul(
                                        av_B[:, ds(qo, w)],
                                        va4[:, tj, 2 * p + 1],
                                        e_B[:, ds(rd, w)],
                                        start=(tj == 0), stop=(tj == nblk - 1))
                            # pace the backlog across the pair's superslots;
                            # for the last pair items arrive per chunk — emit
                            # one per superslot so late superslots stay fed
                            done_slots += 1
                            if p == NPAIR - 1:
                                if emitted < len(backlog):
                                    backlog[emitted]()
                                    emitted += 1
                            else:
                                want = -(-len(backlog) * done_slots // total_slots)
                                if c == 0:
                                    # front-load: chunk 0's S work is tiny and
                                    # its AV waits on the first exps — fill
                                    want += 2
                                while emitted < want and emitted < len(backlog):
                                    backlog[emitted]()
                                    emitted += 1
                        # stage av to SBUF with one copy so the PSUM banks free
                        # early; normalize from the staged copy. The very last
                        # chunk has no successor -> normalize from PSUM
                        # directly, immediately, with both heads' reciprocals
                        # in one op and the broadcast on the (idle) PE.
                        if p == NPAIR - 1 and c == NCH - 1:
                            for head, av in ((0, av_A), (1, av_B)):
                                dn = npool.tile([1, 512], f32, tag="dn",
                                                name=f"dnt{head}")
                                nc.vector.tensor_copy(dn[:], av[D:D + 1, :])
                                rr = npool.tile([1, 512], f32, tag="rr",
                                                name=f"rrt{head}")
                                nc.vector.reciprocal_approx_fast(
                                    out=rr[:], in_=dn[:])
                                rb = npool.tile([D, 512], f32, tag="rb",
                                                name=f"rbt{head}")
                                nc.gpsimd.partition_broadcast(rb[:], rr[:])
                                nc.vector.tensor_mul(
                                    yT[ds(D * head, D), p, ds(c * 512, 512)],
                                    av[0:D, :], rb[:])
                        else:
                            for head, av in ((0, av_A), (1, av_B)):
                                avs = npool.tile([D + 1, 512], f32, tag="avs",
                                                 bufs=4, name=f"avs{p}_{c}_{head}")
                                nc.vector.tensor_copy(avs[:], av[0:D + 1, :])
                                deferred_norm.append(make_norm(avs, head, p, c))

                for fn in deferred_norm:
                    fn()
                deferred_norm.clear()

                # ---- phase 3: projection tail (last query chunk) ----
                for t in range(12, NT):
                    for n2 in range(2):
                        proj_group(t, n2)()

    nc.compile()
    return nc


def _make_mask():
    # causal triangle within a 128x128 diagonal block: mask[p, j] = 1 iff j >= p
    p = np.arange(128)[:, None]
    j = np.arange(128)[None, :]
    return (j >= p).astype(ml_dtypes.bfloat16)


def kernel(x: np.ndarray, W_attn: np.ndarray, W_proj: np.ndarray) -> np.ndarray:
    global LAST_RESULTS
    x = np.asarray(x, dtype=np.float32)
    W_attn = np.asarray(W_attn, dtype=np.float32)
    W_proj = np.asarray(W_proj, dtype=np.float32)

    nc = _cache.get("nc")
    if nc is None:
        nc = _build()
        _cache["nc"] = nc

    mask = _make_mask()
    xTs = [np.ascontiguousarray(x[b].T).astype(ml_dtypes.bfloat16) for b in range(B)]
    in_maps = []
    for c in range(NCORES):
        b, hh = c // 2, c % 2
        qcols = W_attn[:, hh * 512:(hh + 1) * 512]
        kcols = W_attn[:, C + hh * 512:C + (hh + 1) * 512]
        wqk = np.concatenate([qcols, kcols], axis=1)                  # [1024, 1024]
        wqk_blocks = np.ascontiguousarray(
            wqk.reshape(NK, 128, 8, 128).transpose(2, 0, 1, 3)
        ).astype(ml_dtypes.bfloat16)                                  # [m, ko, p, mm]
        wv = np.ascontiguousarray(
            W_attn[:, 2 * C + hh * 512:2 * C + (hh + 1) * 512]
        ).astype(ml_dtypes.bfloat16)
        wp = np.ascontiguousarray(W_proj[hh * 512:(hh + 1) * 512, :]).astype(np.float16)
        in_maps.append({
            "xT": xTs[b], "wqk": wqk_blocks, "wv": wv, "wp": wp, "mask": mask,
        })

    res = run_bass_kernel_spmd(nc, in_maps, core_ids=list(range(NCORES)))
    LAST_RESULTS = res
    parts = [res.results[c]["out"] for c in range(NCORES)]
    out = np.stack([parts[2 * b] + parts[2 * b + 1] for b in range(B)], axis=0)
    return np.ascontiguousarray(out, dtype=np.float32)

